# revision 33
# baseline (speedup 1.0000x reference)
"""Trainium2 Bass kernel for nn_CirculantSTRING.

Math: out[b,n,:] = irfft(exp(i*theta(n,:)) * rfft(x[b,n,:]), n=D)
where theta(n,f) = 2*(p0[n]*Im(rfft(circ0))[f] + p1[n]*Im(rfft(circ1))[f]).

Per core (data-parallel over batch, 4 batches/core), folded real-DFT:
  - even/odd fold: eo = [e_0..e_383 | x_384, o_1..o_383],
    e_d = x_d + x_{768-d}, o_d = x_d - x_{768-d} (halves forward matmul work)
  - forward matmul to fi = [R_0..R_383 | R_384, I_1..I_383]
    (block-sparse F2: 24 of 36 (128,128) blocks)
  - phase rotation with on-device cos/sin(theta) tables; theta(n,0)=0 makes
    the R_384 slot (chunk 3, partition 0) pass through untouched
  - inverse matmul to uv = [u_0..u_384 | v_1..v_383] (7 matmuls/row-group),
    un-fold out[d] = u_d - v_d, out[768-d] = u_d + v_d
Matmuls in float32r (1 cyc/row on PE at N>=256, ~11 mantissa bits); phase
path in fp32.
"""
import math
from contextlib import ExitStack

import numpy as np

import concourse.bacc as bacc
import concourse.tile as tile
from concourse import mybir
from concourse import bass_utils
from concourse.masks import make_identity

F32 = mybir.dt.float32
F32R = mybir.dt.float32r
I32 = mybir.dt.int32

B, N, D = 32, 1024, 768
NCORES = 8
BS = B // NCORES
P = 128
NCH = D // P              # 6
ROWTILE = 512
NG = ROWTILE // P         # 4

TWOPI = 2.0 * math.pi


def _dft_matrices():
    """Host-side DFT constants.

    f_full (768,768): full real-DFT stationary, laid out [d, m] where
      m=0..384 -> R_m rows (cos), m=385..767 -> I_{m-384} rows (-sin).
    g_cp (384,386): inverse u-part rows w_f*cos(2pi f q/768), w_0=1/D
      else 2/D, col 385 zero.
    g_sv (384,384): inverse v-part rows (2/D)sin(2pi f (q+1)/768),
      col 383 zero.
    g_c384 (1,386): (1/D)(-1)^q row for the R_384 slot, col 385 zero.
    ss_c (384,384): sin base matrix for the on-device theta computation.
    """
    d = np.arange(D)
    m = np.arange(D)
    F = np.empty((D, D), dtype=np.float32)
    F[:, 0:385] = np.cos(2 * np.pi * np.outer(d, m[0:385]) / D)
    F[:, 385:768] = -np.sin(2 * np.pi * np.outer(d, m[385:768] - 384) / D)
    f_ = np.arange(384)
    q = np.arange(386)
    w = np.full((384, 1), 2.0 / D, dtype=np.float32)
    w[0, 0] = 1.0 / D
    g_cp = (w * np.cos(2 * np.pi * np.outer(f_, q) / D)).astype(np.float32)
    g_cp[:, 385] = 0.0
    g_sv = ((2.0 / D) * np.sin(2 * np.pi * np.outer(f_, np.arange(1, 385))
                               / D)).astype(np.float32)
    g_sv[:, 383] = 0.0
    g_c384 = ((1.0 / D) * np.cos(2 * np.pi * 384 * q / D)
              ).astype(np.float32).reshape(1, 386)
    g_c384[0, 385] = 0.0
    ds_ = np.arange(384)
    S = np.sin(2 * np.pi * np.outer(ds_, ds_) / D).astype(np.float32)
    return {"f_full": F, "g_cp": g_cp, "g_sv": g_sv, "g_c384": g_c384,
            "ss_c": S}


def build_kernel(mm_dtype=F32R, reps=1, trace_sim=False):
    nc = bacc.Bacc("TRN2", target_bir_lowering=False, debug=False,
                   num_devices=NCORES)
    x = nc.dram_tensor("x", [BS, N, D], F32, kind="ExternalInput").ap()
    circ = nc.dram_tensor("circ", [2, D], F32, kind="ExternalInput").ap()
    positions = nc.dram_tensor("positions", [N, 2], I32,
                               kind="ExternalInput").ap()
    f_full = nc.dram_tensor("f_full", [D, D], F32, kind="ExternalInput").ap()
    g_cp = nc.dram_tensor("g_cp", [384, 386], F32, kind="ExternalInput").ap()
    g_sv = nc.dram_tensor("g_sv", [384, 384], F32, kind="ExternalInput").ap()
    g_c384 = nc.dram_tensor("g_c384", [1, 386], F32,
                            kind="ExternalInput").ap()
    ss_c = nc.dram_tensor("ss_c", [384, 384], F32, kind="ExternalInput").ap()
    out = nc.dram_tensor("out", [BS, N, D], F32, kind="ExternalOutput").ap()

    with tile.TileContext(nc, trace_sim=trace_sim) as tc, ExitStack() as ctx:
        consts = ctx.enter_context(tc.tile_pool(name="consts", bufs=1))
        tabs = ctx.enter_context(tc.tile_pool(name="tabs", bufs=1))
        stage_ctx = ExitStack()
        stage = stage_ctx.enter_context(tc.tile_pool(name="stage", bufs=1))

        ident = consts.tile([P, P], F32, tag="ident")
        make_identity(nc, ident)

        ps0 = tc.tile_pool(name="ps0", bufs=1, space="PSUM")
        psum = ps0.__enter__()
        hp = tc.high_priority()
        hp.__enter__()

        # ---- circ odd-fold (for s2, in fp32) ----
        circR = tabs.tile([2, D], F32, tag="circR")
        nc.sync.dma_start(out=circR, in_=circ)
        ocr = tabs.tile([2, 384], F32, tag="ocr")
        nc.vector.memset(ocr[:, 0:1], 0.0)
        nc.vector.tensor_sub(ocr[:, 1:384], circR[:, 1:384],
                             circR[:, 767:384:-1])
        occ = []  # (128, 2) fp32, o-fold of circ on chunk 3..5 partitions
        for i in range(3):
            poc = psum.tile([P, 2], F32, tag="pocc")
            nc.tensor.transpose(poc, ocr[:, i * P:(i + 1) * P], ident[0:2, 0:2])
            so = tabs.tile([P, 2], F32, tag=f"occ{i}")
            nc.scalar.copy(out=so, in_=poc)
            occ.append(so)

        # ---- load S base matrix (theta path) ----
        Sst = []
        for i in range(3):
            t_s = stage.tile([P, 384], F32, tag=f"sst{i}", name=f"sst{i}")
            nc.sync.dma_start(out=t_s, in_=ss_c[i * P:(i + 1) * P, :])
            Sst.append(t_s)

        # s2' = sum_i occ[i]^T @ S-chunk (theta sign absorbed into posTf)
        s2ps = psum.tile([2, 384], F32, tag="s2ps")
        for i in range(3):
            nc.tensor.matmul(s2ps[:, 1:384], occ[i], Sst[i][:, 1:384],
                             start=(i == 0), stop=(i == 2))
        s2 = tabs.tile([2, 384], F32, tag="s2")
        nc.vector.memset(s2[:, 0:1], 0.0)
        nc.vector.tensor_copy(out=s2[:, 1:384], in_=s2ps[:, 1:384])

        # ---- forward/inverse stationaries, host-precomputed, staged via
        # fp32 then rounded to f32r by Act copies (f32r matmul inputs need
        # a rounding producer) ----
        FPt = []
        for c in range(NCH):
            fst = stage.tile([P, D], F32, tag="fst", name=f"fst{c}")
            nc.sync.dma_start(out=fst, in_=f_full[c * P:(c + 1) * P, :])
            t = consts.tile([P, D], mm_dtype, tag=f"fp{c}", name=f"fp{c}")
            nc.scalar.copy(out=t, in_=fst)
            FPt.append(t)
        # inverse stationaries for the merged-rotation inverse: pa (u-part)
        # consumes t1 via +C and t2 via -C, plus the R_384 slot (c384 row);
        # pb (v-part) consumes t3 and t4 via +S.  (u = C@(t1-t2)+c384*R384,
        # v = S@(t3+t4); rotation combines fold into the matmul signs.)
        Cp, Cn, Sv = [], [], []
        for jj in range(3):
            gst = stage.tile([P, 386], F32, tag="gst", name=f"gcp{jj}")
            nc.sync.dma_start(out=gst, in_=g_cp[jj * P:(jj + 1) * P, :])
            t = consts.tile([P, 386], mm_dtype, tag=f"cp{jj}", name=f"cp{jj}")
            nc.scalar.copy(out=t, in_=gst)
            Cp.append(t)
            tn = consts.tile([P, 386], mm_dtype, tag=f"cn{jj}",
                             name=f"cn{jj}")
            nc.scalar.mul(out=tn, in_=gst, mul=-1.0)
            Cn.append(tn)
        for jj in range(3):
            gst = stage.tile([P, 384], F32, tag="gsst", name=f"gsv{jj}")
            nc.sync.dma_start(out=gst, in_=g_sv[jj * P:(jj + 1) * P, :])
            t = consts.tile([P, 384], mm_dtype, tag=f"sv{jj}", name=f"sv{jj}")
            nc.scalar.copy(out=t, in_=gst)
            Sv.append(t)
        g384s = stage.tile([1, 386], F32, tag="g384s")
        nc.sync.dma_start(out=g384s, in_=g_c384)
        c384r = consts.tile([1, 386], mm_dtype, tag="c384r")
        nc.scalar.copy(out=c384r, in_=g384s)

        # ---- positions ----
        posT = tabs.tile([2, N], I32, tag="posT")
        nc.sync.dma_start(out=posT, in_=positions.rearrange("n k -> k n"))
        posTf = tabs.tile([2, N], F32, tag="posTf")
        nc.vector.tensor_scalar_mul(posTf, posT, -2.0)

        # ---- theta -> cos/sin tables (3 chunks of (128, N)) ----
        cT, sT = [], []
        for j in range(3):
            thps = psum.tile([P, N], F32, tag="thps")
            for h in range(2):
                nc.tensor.matmul(thps[:, h * 512:(h + 1) * 512],
                                 s2[:, j * P:(j + 1) * P],
                                 posTf[:, h * 512:(h + 1) * 512],
                                 start=True, stop=True)
            sj = [tabs.tile([P, 512], F32, tag=f"sT{j}_{hh}",
                            name=f"sT{j}_{hh}") for hh in range(2)]
            cj = [tabs.tile([P, 512], F32, tag=f"cT{j}_{hh}",
                            name=f"cT{j}_{hh}") for hh in range(2)]
            for hh in range(2):
                hs = slice(hh * 512, (hh + 1) * 512)
                te = stage.tile([P, 512], F32, tag="te")
                nc.scalar.copy(out=te, in_=thps[:, hs])
                t1 = stage.tile([P, 512], F32, tag="pt")
                r1 = stage.tile([P, 512], I32, tag="pr")
                u1 = stage.tile([P, 512], F32, tag="pu")
                red = stage.tile([P, 512], F32, tag="pred")
                nc.vector.tensor_scalar_mul(t1, te, 1.0 / TWOPI)
                nc.vector.tensor_copy(out=r1, in_=t1)
                nc.vector.tensor_scalar_mul(u1, r1, -TWOPI)
                nc.vector.tensor_add(red, te, u1)
                nc.scalar.activation(out=sj[hh], in_=red,
                                     func=mybir.ActivationFunctionType.Sin)
                t2 = stage.tile([P, 512], F32, tag="qt")
                r2 = stage.tile([P, 512], I32, tag="qr")
                u2 = stage.tile([P, 512], F32, tag="qu")
                red2 = stage.tile([P, 512], F32, tag="qred")
                nc.gpsimd.tensor_scalar(t2, te, 1.0 / TWOPI, 0.25,
                                        op0=mybir.AluOpType.mult,
                                        op1=mybir.AluOpType.add)
                nc.vector.tensor_copy(out=r2, in_=t2)
                nc.gpsimd.tensor_scalar(u2, r2, -TWOPI, math.pi / 2,
                                        op0=mybir.AluOpType.mult,
                                        op1=mybir.AluOpType.add)
                nc.gpsimd.tensor_add(red2, te, u2)
                nc.scalar.activation(out=cj[hh], in_=red2,
                                     func=mybir.ActivationFunctionType.Sin)
            sT.append(sj)
            cT.append(cj)
        hp.__exit__(None, None, None)
        ps0.__exit__(None, None, None)

        # ---- main loop ----
        stage_ctx.close()  # reclaim setup staging SBUF for xio/work
        xio = ctx.enter_context(tc.tile_pool(name="xio", bufs=2))
        work = ctx.enter_context(tc.tile_pool(name="work", bufs=2))
        # PSUM: 8 banks total = pst*2 + pR*2 + pI*2 + pa + pb
        pst_pool = ctx.enter_context(tc.tile_pool(name="pst", bufs=2,
                                                  space="PSUM"))
        psf = ctx.enter_context(tc.tile_pool(name="psf", bufs=2, space="PSUM"))
        psi = ctx.enter_context(tc.tile_pool(name="psi", bufs=1, space="PSUM"))

        tiles = [(b, h) for b in range(BS) for h in range(2)]

        def load(i):
            b, h = tiles[i]
            xh = xio.tile([P, NG * D], F32, tag="xh", name=f"xh{i % 2}")
            nc.sync.dma_start(
                out=xh.rearrange("p (g d) -> p g d", g=NG),
                in_=x[b, h * ROWTILE:(h + 1) * ROWTILE, :].rearrange(
                    "(g p) d -> p g d", g=NG))
            return xh

        rep_ctx = tc.For_i(0, reps, 1) if reps > 1 else None
        if rep_ctx is not None:
            rep_ctx.__enter__()
        xh_next = None
        for i in range(len(tiles)):
            b, h = tiles[i]
            xh = xh_next if xh_next is not None else load(i)
            xh_next = load(i + 1) if i + 1 < len(tiles) else None
            # transpose raw x to (d, rows): 6 chunks of (128, 512); the
            # even/odd fold lives inside the full f_full stationary instead
            XT = []
            for c in range(NCH):
                pst = pst_pool.tile([P, ROWTILE], F32, tag="pst")
                for g in range(NG):
                    nc.tensor.transpose(pst[:, g * P:(g + 1) * P],
                                        xh[:, g * D + c * P:
                                            g * D + (c + 1) * P], ident)
                xt = work.tile([P, ROWTILE], mm_dtype, tag=f"xt{c}")
                nc.scalar.copy(out=xt, in_=pst)
                XT.append(xt)
            # forward (block-sparse) + rotation products per pair (j, 3+j);
            # the t1..t4 products feed the inverse directly (combine signs
            # are baked into the Cp/Cn/Sv stationaries)
            T1, T2, T3, T4 = [], [], [], []
            for j in range(3):
                pR = psf.tile([P, ROWTILE], F32, tag="pR")
                pI = psf.tile([P, ROWTILE], F32, tag="pI")
                for c in range(NCH):
                    nc.tensor.matmul(pR, FPt[c][:, j * P:(j + 1) * P],
                                     XT[c], start=(c == 0),
                                     stop=(c == NCH - 1))
                for c in range(NCH):
                    nc.tensor.matmul(pI,
                                     FPt[c][:, (3 + j) * P:(4 + j) * P],
                                     XT[c], start=(c == 0),
                                     stop=(c == NCH - 1))
                cs = cT[j][h]
                sn = sT[j][h]
                # pI consumers first so its bank frees for fwd j+1
                t2 = work.tile([P, ROWTILE], mm_dtype, tag=f"t2_{j}",
                               name=f"t2_{j}")
                t4 = work.tile([P, ROWTILE], mm_dtype, tag=f"t4_{j}",
                               name=f"t4_{j}")
                t1 = work.tile([P, ROWTILE], mm_dtype, tag=f"t1_{j}",
                               name=f"t1_{j}")
                t3 = work.tile([P, ROWTILE], mm_dtype, tag=f"t3_{j}",
                               name=f"t3_{j}")
                nc.vector.tensor_mul(t2, pI, sn)
                nc.vector.tensor_mul(t4, pI, cs)
                nc.vector.tensor_mul(t1, pR, cs)
                nc.vector.tensor_mul(t3, pR, sn)
                T1.append(t1)
                T2.append(t2)
                T3.append(t3)
                T4.append(t4)
            # inverse (folded, rotation-combines merged): u (386) and
            # v (384) psum, batched un-fold to osb
            osb = xio.tile([P, NG * D], F32, tag="osb")
            ua_all = work.tile([P, NG * 386], F32, tag="ua", bufs=1)
            vb_all = work.tile([P, NG * 384], F32, tag="vb", bufs=1)
            for g in range(NG):
                pa = psi.tile([P, 386], F32, tag="pa")
                pb = psi.tile([P, 384], F32, tag="pb")
                gs = slice(g * P, (g + 1) * P)
                mma = ([(T1[j][:, gs], Cp[j]) for j in range(3)]
                       + [(T2[j][:, gs], Cn[j]) for j in range(3)]
                       + [(T4[0][0:1, gs], c384r)])
                for idx, (lhsT, rhs) in enumerate(mma):
                    nc.tensor.matmul(pa, lhsT, rhs, start=(idx == 0),
                                     stop=(idx == len(mma) - 1))
                mmb = ([(T3[j][:, gs], Sv[j]) for j in range(3)]
                       + [(T4[j][:, gs], Sv[j]) for j in range(3)])
                for idx, (lhsT, rhs) in enumerate(mmb):
                    nc.tensor.matmul(pb, lhsT, rhs, start=(idx == 0),
                                     stop=(idx == len(mmb) - 1))
                nc.scalar.copy(out=ua_all[:, g * 386:(g + 1) * 386], in_=pa)
                nc.scalar.copy(out=vb_all[:, g * 384:(g + 1) * 384], in_=pb)
            ua3 = ua_all.rearrange("p (g d) -> p g d", g=NG)
            vb3 = vb_all.rearrange("p (g d) -> p g d", g=NG)
            osb3 = osb.rearrange("p (g d) -> p g d", g=NG)
            nc.gpsimd.tensor_sub(osb3[:, :, 1:384], ua3[:, :, 1:384],
                                 vb3[:, :, 0:383])
            nc.gpsimd.tensor_add(osb3[:, :, 385:768], ua3[:, :, 383:0:-1],
                                 vb3[:, :, 382::-1])
            nc.vector.tensor_copy(out=osb3[:, :, 0:385:384],
                                  in_=ua3[:, :, 0:385:384])
            nc.sync.dma_start(
                out=out[b, h * ROWTILE:(h + 1) * ROWTILE, :].rearrange(
                    "(g p) d -> p g d", g=NG),
                in_=osb.rearrange("p (g d) -> p g d", g=NG))
        if rep_ctx is not None:
            rep_ctx.__exit__(None, None, None)
    nc.finalize()
    return nc


_NC_CACHE = {}


def kernel(x, circ, positions):
    x = np.ascontiguousarray(x, dtype=np.float32)
    circ = np.ascontiguousarray(circ, dtype=np.float32)
    positions = np.ascontiguousarray(positions, dtype=np.int32)
    if "nc" not in _NC_CACHE:
        _NC_CACHE["nc"] = build_kernel()
    nc = _NC_CACHE["nc"]
    consts = _dft_matrices()
    in_maps = []
    for core in range(NCORES):
        in_maps.append({
            "x": x[core * BS:(core + 1) * BS],
            "circ": circ,
            "positions": positions,
            **consts,
        })
    res = bass_utils.run_bass_kernel_spmd(nc, in_maps,
                                          core_ids=list(range(NCORES)))
    out = np.concatenate([res.results[c]["out"] for c in range(NCORES)],
                         axis=0)
    return out


if __name__ == "__main__":
    rng = np.random.default_rng(0)
    x = rng.standard_normal((B, N, D)).astype(np.float32)
    circ = (rng.standard_normal((2, D)) * 0.01).astype(np.float32)
    positions = rng.integers(0, 32, (N, 2)).astype(np.int32)
    out = kernel(x=x, circ=circ, positions=positions)
    print("out", out.shape, out.dtype)



# revision 34
# speedup vs baseline: 1.0337x; 1.0337x over previous
"""Trainium2 Bass kernel for nn_CirculantSTRING.

Math: out[b,n,:] = irfft(exp(i*theta(n,:)) * rfft(x[b,n,:]), n=D)
where theta(n,f) = 2*(p0[n]*Im(rfft(circ0))[f] + p1[n]*Im(rfft(circ1))[f]).

Per core (data-parallel over batch, 4 batches/core), folded real-DFT:
  - even/odd fold: eo = [e_0..e_383 | x_384, o_1..o_383],
    e_d = x_d + x_{768-d}, o_d = x_d - x_{768-d} (halves forward matmul work)
  - forward matmul to fi = [R_0..R_383 | R_384, I_1..I_383]
    (block-sparse F2: 24 of 36 (128,128) blocks)
  - phase rotation with on-device cos/sin(theta) tables; theta(n,0)=0 makes
    the R_384 slot (chunk 3, partition 0) pass through untouched
  - inverse matmul to uv = [u_0..u_384 | v_1..v_383] (7 matmuls/row-group),
    un-fold out[d] = u_d - v_d, out[768-d] = u_d + v_d
Matmuls in float32r (1 cyc/row on PE at N>=256, ~11 mantissa bits); phase
path in fp32.  Single 512-row DMAs; f32r identity halves nothing but cuts
transpose cost 2.0->1.5 cyc/row; PSUM split pst*2/pR*2/pI*2/pa/pb.
"""
import math
from contextlib import ExitStack

import numpy as np

import concourse.bacc as bacc
import concourse.tile as tile
from concourse import mybir
from concourse import bass_utils
from concourse.masks import make_identity

F32 = mybir.dt.float32
F32R = mybir.dt.float32r
I32 = mybir.dt.int32

B, N, D = 32, 1024, 768
NCORES = 8
BS = B // NCORES
P = 128
NCH = D // P              # 6
ROWTILE = 512
NG = ROWTILE // P         # 4

TWOPI = 2.0 * math.pi

# forward block list: M-chunk -> list of K-chunks
FWD_BLOCKS = {0: [0, 1, 2, 3], 1: [0, 1, 2, 3], 2: [0, 1, 2, 3],
              3: [0, 1, 2, 3, 4, 5], 4: [3, 4, 5], 5: [3, 4, 5]}


def _dft_matrices():
    """Symmetric base matrices: C (385,385) cos incl boundary row/col,
    S (384,384) sin (row/col 0 are zero)."""
    dc = np.arange(385)
    C = np.cos(2 * np.pi * np.outer(dc, dc) / D).astype(np.float32)
    ds_ = np.arange(384)
    S = np.sin(2 * np.pi * np.outer(ds_, ds_) / D).astype(np.float32)
    return C, S


def build_kernel(mm_dtype=F32R, reps=1, trace_sim=False):
    nc = bacc.Bacc("TRN2", target_bir_lowering=False, debug=False,
                   num_devices=NCORES)
    x = nc.dram_tensor("x", [BS, N, D], F32, kind="ExternalInput").ap()
    circ = nc.dram_tensor("circ", [2, D], F32, kind="ExternalInput").ap()
    positions = nc.dram_tensor("positions", [N, 2], I32,
                               kind="ExternalInput").ap()
    cs_c = nc.dram_tensor("cs_c", [385, 385], F32, kind="ExternalInput").ap()
    ss_c = nc.dram_tensor("ss_c", [384, 384], F32, kind="ExternalInput").ap()
    out = nc.dram_tensor("out", [BS, N, D], F32, kind="ExternalOutput").ap()

    with tile.TileContext(nc, trace_sim=trace_sim) as tc, ExitStack() as ctx:
        consts = ctx.enter_context(tc.tile_pool(name="consts", bufs=1))
        tabs = ctx.enter_context(tc.tile_pool(name="tabs", bufs=1))
        stage_ctx = ExitStack()
        stage = stage_ctx.enter_context(tc.tile_pool(name="stage", bufs=1))

        ident = consts.tile([P, P], F32, tag="ident")
        make_identity(nc, ident)
        # transpose cost is set by the MOVING operand (the identity):
        # f32r streams 1.5 cyc/row vs fp32's 2.0, bit-identical values.
        identR = consts.tile([P, P], F32R, tag="identR")
        nc.scalar.copy(out=identR, in_=ident)

        ps0 = tc.tile_pool(name="ps0", bufs=1, space="PSUM")
        psum = ps0.__enter__()
        hp = tc.high_priority()
        hp.__enter__()

        # ---- circ odd-fold (for s2, in fp32) ----
        circR = tabs.tile([2, D], F32, tag="circR")
        nc.sync.dma_start(out=circR, in_=circ)
        ocr = tabs.tile([2, 384], F32, tag="ocr")
        nc.vector.memset(ocr[:, 0:1], 0.0)
        nc.vector.tensor_sub(ocr[:, 1:384], circR[:, 1:384],
                             circR[:, 767:384:-1])
        occ = []  # (128, 2) fp32, o-fold of circ on chunk 3..5 partitions
        for i in range(3):
            poc = psum.tile([P, 2], F32, tag="pocc")
            nc.tensor.transpose(poc, ocr[:, i * P:(i + 1) * P], ident[0:2, 0:2])
            so = tabs.tile([P, 2], F32, tag=f"occ{i}")
            nc.scalar.copy(out=so, in_=poc)
            occ.append(so)

        # ---- load C/S base matrices, assemble F2/G2 tiles, s2 matmul ----
        Cst, Sst = [], []
        for i in range(3):
            t_s = stage.tile([P, 384], F32, tag=f"sst{i}", name=f"sst{i}")
            nc.sync.dma_start(out=t_s, in_=ss_c[i * P:(i + 1) * P, :])
            Sst.append(t_s)
        for i in range(3):
            t_c = stage.tile([P, 385], F32, tag=f"cst{i}", name=f"cst{i}")
            nc.sync.dma_start(out=t_c, in_=cs_c[i * P:(i + 1) * P, :])
            Cst.append(t_c)
        c384 = stage.tile([1, 385], F32, tag="c384")
        nc.sync.dma_start(out=c384, in_=cs_c[384:385, :])

        # s2' = sum_i occ[i]^T @ S-chunk (theta sign absorbed into posTf)
        s2ps = psum.tile([2, 384], F32, tag="s2ps")
        for i in range(3):
            nc.tensor.matmul(s2ps[:, 1:384], occ[i], Sst[i][:, 1:384],
                             start=(i == 0), stop=(i == 2))
        s2 = tabs.tile([2, 384], F32, tag="s2")
        nc.vector.memset(s2[:, 0:1], 0.0)
        nc.vector.tensor_copy(out=s2[:, 1:384], in_=s2ps[:, 1:384])

        # per-partition inverse scales: wv = 2/768 (p0 of chunk0 -> 1/768)
        wv = consts.tile([P, 1], F32, tag="wv")
        nc.vector.memset(wv, 2.0 / D)
        wv0 = consts.tile([P, 1], F32, tag="wv0")
        nc.vector.memset(wv0, 2.0 / D)
        nc.vector.memset(wv0[0:1, :], 1.0 / D)

        FPt, GPt = [], []
        for c in range(NCH):
            t = consts.tile([P, D], mm_dtype, tag=f"fp{c}", name=f"fp{c}")
            if c <= 2:
                nc.scalar.copy(out=t[:, 0:385], in_=Cst[c])
                nc.gpsimd.memset(t[:, 385:768].bitcast(F32), 0.0)
            elif c == 3:
                nc.scalar.mul(out=t[:, 385:768], in_=Sst[0][:, 1:384],
                              mul=-1.0)  # row 0 of S is zero
                nc.gpsimd.memset(t[:, 0:385].bitcast(F32), 0.0)
                nc.scalar.copy(out=t[0:1, 0:385], in_=c384)
            else:
                nc.gpsimd.memset(t[:, 0:385].bitcast(F32), 0.0)
                nc.scalar.mul(out=t[:, 385:768], in_=Sst[c - 3][:, 1:384],
                              mul=-1.0)
            FPt.append(t)
        for c in range(NCH):
            t = consts.tile([P, 770], mm_dtype, tag=f"gp{c}", name=f"gp{c}")
            if c <= 2:
                nc.scalar.mul(out=t[:, 0:385], in_=Cst[c],
                              mul=(wv0 if c == 0 else wv))
                nc.gpsimd.memset(t[:, 385:770].bitcast(F32), 0.0)
            elif c == 3:
                nc.scalar.mul(out=t[:, 386:769], in_=Sst[0][:, 1:384],
                              mul=2.0 / D)  # row 0 of S is zero
                nc.gpsimd.memset(t[:, 0:386].bitcast(F32), 0.0)
                nc.gpsimd.memset(t[:, 769:770].bitcast(F32), 0.0)
                nc.scalar.mul(out=t[0:1, 0:385], in_=c384, mul=1.0 / D)
            else:
                nc.gpsimd.memset(t[:, 0:386].bitcast(F32), 0.0)
                nc.scalar.mul(out=t[:, 386:769], in_=Sst[c - 3][:, 1:384],
                              mul=2.0 / D)
                nc.gpsimd.memset(t[:, 769:770].bitcast(F32), 0.0)
            GPt.append(t)

        # ---- positions ----
        posT = tabs.tile([2, N], I32, tag="posT")
        nc.sync.dma_start(out=posT, in_=positions.rearrange("n k -> k n"))
        posTf = tabs.tile([2, N], F32, tag="posTf")
        nc.vector.tensor_scalar_mul(posTf, posT, -2.0)

        # ---- theta -> cos/sin tables (3 chunks of (128, N)) ----
        cT, sT = [], []
        for j in range(3):
            thps = psum.tile([P, N], F32, tag="thps")
            for h in range(2):
                nc.tensor.matmul(thps[:, h * 512:(h + 1) * 512],
                                 s2[:, j * P:(j + 1) * P],
                                 posTf[:, h * 512:(h + 1) * 512],
                                 start=True, stop=True)
            sj = [tabs.tile([P, 512], F32, tag=f"sT{j}_{hh}",
                            name=f"sT{j}_{hh}") for hh in range(2)]
            cj = [tabs.tile([P, 512], F32, tag=f"cT{j}_{hh}",
                            name=f"cT{j}_{hh}") for hh in range(2)]
            for hh in range(2):
                hs = slice(hh * 512, (hh + 1) * 512)
                te = stage.tile([P, 512], F32, tag="te")
                nc.scalar.copy(out=te, in_=thps[:, hs])
                t1 = stage.tile([P, 512], F32, tag="pt")
                r1 = stage.tile([P, 512], I32, tag="pr")
                u1 = stage.tile([P, 512], F32, tag="pu")
                red = stage.tile([P, 512], F32, tag="pred")
                nc.vector.tensor_scalar_mul(t1, te, 1.0 / TWOPI)
                nc.vector.tensor_copy(out=r1, in_=t1)
                nc.vector.tensor_scalar_mul(u1, r1, -TWOPI)
                nc.vector.tensor_add(red, te, u1)
                nc.scalar.activation(out=sj[hh], in_=red,
                                     func=mybir.ActivationFunctionType.Sin)
                t2 = stage.tile([P, 512], F32, tag="qt")
                r2 = stage.tile([P, 512], I32, tag="qr")
                u2 = stage.tile([P, 512], F32, tag="qu")
                red2 = stage.tile([P, 512], F32, tag="qred")
                nc.gpsimd.tensor_scalar(t2, te, 1.0 / TWOPI, 0.25,
                                        op0=mybir.AluOpType.mult,
                                        op1=mybir.AluOpType.add)
                nc.vector.tensor_copy(out=r2, in_=t2)
                nc.gpsimd.tensor_scalar(u2, r2, -TWOPI, math.pi / 2,
                                        op0=mybir.AluOpType.mult,
                                        op1=mybir.AluOpType.add)
                nc.gpsimd.tensor_add(red2, te, u2)
                nc.scalar.activation(out=cj[hh], in_=red2,
                                     func=mybir.ActivationFunctionType.Sin)
            sT.append(sj)
            cT.append(cj)
        hp.__exit__(None, None, None)
        ps0.__exit__(None, None, None)

        # ---- main loop ----
        stage_ctx.close()  # reclaim setup staging SBUF for xio/work
        xio = ctx.enter_context(tc.tile_pool(name="xio", bufs=2))
        work = ctx.enter_context(tc.tile_pool(name="work", bufs=2))
        # PSUM: 8 banks total = pst*2 + pR*2 + pI*2 + pa + pb
        pst_pool = ctx.enter_context(tc.tile_pool(name="pst", bufs=2,
                                                  space="PSUM"))
        psf = ctx.enter_context(tc.tile_pool(name="psf", bufs=2, space="PSUM"))
        psi = ctx.enter_context(tc.tile_pool(name="psi", bufs=1, space="PSUM"))

        tiles = [(b, h) for b in range(BS) for h in range(2)]

        def load(i):
            b, h = tiles[i]
            xh = xio.tile([P, NG * D], F32, tag="xh", name=f"xh{i % 2}")
            nc.sync.dma_start(
                out=xh.rearrange("p (g d) -> p g d", g=NG),
                in_=x[b, h * ROWTILE:(h + 1) * ROWTILE, :].rearrange(
                    "(g p) d -> p g d", g=NG))
            return xh

        rep_ctx = tc.For_i(0, reps, 1) if reps > 1 else None
        if rep_ctx is not None:
            rep_ctx.__enter__()
        xh_next = None
        for i in range(len(tiles)):
            b, h = tiles[i]
            xh = xh_next if xh_next is not None else load(i)
            xh_next = load(i + 1) if i + 1 < len(tiles) else None
            # even/odd fold on Pool (boundary cols on DVE), fp32r out
            eog = []
            for g in range(NG):
                eo = xio.tile([P, D], F32R, tag=f"eo{g}", name=f"eo{g}")
                g0 = g * D
                nc.gpsimd.tensor_add(eo[:, 1:384], xh[:, g0 + 1:g0 + 384],
                                     xh[:, g0 + 767:g0 + 384:-1])
                nc.gpsimd.tensor_sub(eo[:, 385:768], xh[:, g0 + 1:g0 + 384],
                                     xh[:, g0 + 767:g0 + 384:-1])
                nc.vector.tensor_copy(out=eo[:, 0:385:384],
                                      in_=xh[:, g0:g0 + 385:384])
                eog.append(eo)
            # transpose eo to (d', rows): 6 chunks of (128, 512)
            XT = []
            for c in range(NCH):
                pst = pst_pool.tile([P, ROWTILE], F32R, tag="pst")
                for g in range(NG):
                    nc.tensor.transpose(pst[:, g * P:(g + 1) * P],
                                        eog[g][:, c * P:(c + 1) * P], identR)
                xt = work.tile([P, ROWTILE], mm_dtype, tag=f"xt{c}")
                nc.scalar.copy(out=xt, in_=pst)
                XT.append(xt)
            # forward (block-sparse) + rotation per pair (j, 3+j)
            RI = [None] * NCH
            for j in range(3):
                pR = psf.tile([P, ROWTILE], F32, tag="pR")
                pI = psf.tile([P, ROWTILE], F32, tag="pI")
                kR = FWD_BLOCKS[j]
                for idx, c in enumerate(kR):
                    nc.tensor.matmul(pR, FPt[c][:, j * P:(j + 1) * P],
                                     XT[c], start=(idx == 0),
                                     stop=(idx == len(kR) - 1))
                kI = FWD_BLOCKS[3 + j]
                for idx, c in enumerate(kI):
                    nc.tensor.matmul(pI,
                                     FPt[c][:, (3 + j) * P:(4 + j) * P],
                                     XT[c], start=(idx == 0),
                                     stop=(idx == len(kI) - 1))
                cs = cT[j][h]
                sn = sT[j][h]
                # pI consumers first so its bank frees for fwd j+1
                t2 = work.tile([P, ROWTILE], F32, tag="rt2", bufs=1)
                t4 = work.tile([P, ROWTILE], F32, tag="rt4", bufs=1)
                t1 = work.tile([P, ROWTILE], F32, tag="rt1", bufs=1)
                t3 = work.tile([P, ROWTILE], F32, tag="rt3", bufs=1)
                nc.vector.tensor_mul(t2, pI, sn)
                nc.vector.tensor_mul(t4, pI, cs)
                nc.vector.tensor_mul(t1, pR, cs)
                nc.vector.tensor_mul(t3, pR, sn)
                rp = work.tile([P, ROWTILE], mm_dtype, tag=f"ri{j}")
                ip = work.tile([P, ROWTILE], mm_dtype, tag=f"ri{3 + j}")
                nc.gpsimd.tensor_sub(rp, t1, t2)
                nc.gpsimd.tensor_add(ip, t3, t4)
                RI[j] = rp
                RI[3 + j] = ip
            # inverse (folded): u (386) and v (384) psum, un-fold to osb
            osb = xio.tile([P, NG * D], F32, tag="osb")
            for g in range(NG):
                pa = psi.tile([P, 386], F32, tag="pa")
                pb = psi.tile([P, 384], F32, tag="pb")
                gs = slice(g * P, (g + 1) * P)
                for idx, c in enumerate((0, 1, 2, 3)):
                    nc.tensor.matmul(pa, RI[c][:, gs], GPt[c][:, 0:386],
                                     start=(idx == 0), stop=(idx == 3))
                for idx, c in enumerate((3, 4, 5)):
                    nc.tensor.matmul(pb, RI[c][:, gs], GPt[c][:, 386:770],
                                     start=(idx == 0), stop=(idx == 2))
                ua = work.tile([P, 386], F32, tag="ua", bufs=1)
                vb = work.tile([P, 384], F32, tag="vb", bufs=1)
                nc.scalar.copy(out=ua, in_=pa)
                nc.scalar.copy(out=vb, in_=pb)
                g0 = g * D
                nc.gpsimd.tensor_sub(osb[:, g0 + 1:g0 + 384], ua[:, 1:384],
                                     vb[:, 0:383])
                nc.gpsimd.tensor_add(osb[:, g0 + 385:g0 + 768],
                                     ua[:, 383:0:-1], vb[:, 382::-1])
                nc.vector.tensor_copy(out=osb[:, g0:g0 + 385:384],
                                      in_=ua[:, 0:385:384])
            nc.sync.dma_start(
                out=out[b, h * ROWTILE:(h + 1) * ROWTILE, :].rearrange(
                    "(g p) d -> p g d", g=NG),
                in_=osb.rearrange("p (g d) -> p g d", g=NG))
        if rep_ctx is not None:
            rep_ctx.__exit__(None, None, None)
    nc.finalize()
    return nc


_NC_CACHE = {}


def kernel(x, circ, positions):
    x = np.ascontiguousarray(x, dtype=np.float32)
    circ = np.ascontiguousarray(circ, dtype=np.float32)
    positions = np.ascontiguousarray(positions, dtype=np.int32)
    if "nc" not in _NC_CACHE:
        _NC_CACHE["nc"] = build_kernel()
    nc = _NC_CACHE["nc"]
    FP, GP = _dft_matrices()
    in_maps = []
    for core in range(NCORES):
        in_maps.append({
            "x": x[core * BS:(core + 1) * BS],
            "circ": circ,
            "positions": positions,
            "cs_c": FP,
            "ss_c": GP,
        })
    res = bass_utils.run_bass_kernel_spmd(nc, in_maps,
                                          core_ids=list(range(NCORES)))
    out = np.concatenate([res.results[c]["out"] for c in range(NCORES)],
                         axis=0)
    return out


if __name__ == "__main__":
    rng = np.random.default_rng(0)
    x = rng.standard_normal((B, N, D)).astype(np.float32)
    circ = (rng.standard_normal((2, D)) * 0.01).astype(np.float32)
    positions = rng.integers(0, 32, (N, 2)).astype(np.int32)
    out = kernel(x=x, circ=circ, positions=positions)
    print("out", out.shape, out.dtype)
